# revision 38
# baseline (speedup 1.0000x reference)
"""GraphConv (DGL norm='both') + log_softmax on 8 Trainium2 NeuronCores.

Strategy (per sharding hint): partition nodes across the 8 cores by range.
  Launch A (per core): project its 12500-node slice m = (h @ W) * out_deg^-1/2
  in bf16, staged as one [128, NG*64] tile, chunked loads/stores for overlap.
  Host: quantize m to fp8e4m3 and lay it out as 4 sub-tables (int16 gather
  index limit) with 256B row stride / 64B payload.
  Launch B (per core): for its 12500 dst nodes, dma_gather the fp8 rows of
  all in-edge sources (64B payload per descriptor), build narrow bf16
  one-hot selectors per 128-row chunk (edges pre-sorted by dst slot), and
  segment-sum via mixed bf16xfp8 matmuls accumulating into per-window PSUM
  partition ranges; then norm/bias/log_softmax, single bf16 store.

Degrees, norms and all edge metadata (sorting, capacities, window/job
structure) are sharding-prep computed on the host (numpy); all FLOPs on
h/W/b/m run on device.
"""

import numpy as np
import ml_dtypes

import concourse.bass as bass
import concourse.bacc as bacc
import concourse.mybir as mybir
import concourse.tile as tile
from concourse import ap_utils
from concourse.bass import AP, MemorySpace, round_up_to_multiple, exact_div
from concourse.bass_utils import run_bass_kernel_spmd

P = 128
N_NODES = 100000
N_EDGES = 3200000
IN_DIM = 256
OUT_DIM = 64
NCORES = 8
G = N_NODES // NCORES            # 12500 nodes per core
NG = (G + P - 1) // P            # 98 groups of 128 dst nodes (last has 84)
GPAD = NG * P                    # 12544
NT = 4                           # gather sub-tables (int16 index limit)
TROWS = (NCORES * GPAD) // NT    # 25088 rows per sub-table
TSTRIDE = 256                    # fp8 bytes per table row (256B stride req)
# round sizes: small first/last rounds shrink pipeline startup/tail; the
# middle rounds amortize the per-gather SWDGE fixed cost
ROUND_SIZES = [2, 4] + [8] * 11 + [2, 1, 1]
assert sum(ROUND_SIZES) == 98
WIN = 32                         # dst-slot window width (psum partition tile)
NWIN = P // WIN
PAD_SLOT = 200.0                 # slot value for padded rows (no window match)

_f32 = mybir.dt.float32
_bf16 = mybir.dt.bfloat16
_f8 = mybir.dt.float8e4
_i16 = mybir.dt.int16


def _expand_mid(ap, n):
    """[P, C] AP -> [P, n, C] AP repeating each partition row n times
    (middle broadcast keeps the last dim packed, so DVE 2x mode applies)."""
    (ps, pc), (cs, cc) = ap.ap[0], ap.ap[1]
    return AP(ap.tensor, ap.offset, [[ps, pc], [0, n], [cs, cc]])


def dma_gather_sub256(g, out_ap, in_ap, idxs_ap, num_idxs, elem_size,
                      elem_step):
    """bass.BassGpSimd.dma_gather (non-transpose, DRAM src, gen_mode=0) with
    payload < 256B: only the table row stride must be a 256B multiple (the
    descriptor stride field is encoded in 256B units); elem_size is a free
    byte count. Verified bit-exact on device for 64B/128B payloads."""
    assert idxs_ap.dtype == mybir.dt.int16
    assert in_ap.dtype == out_ap.dtype
    assert in_ap.space == MemorySpace.DRAM
    assert idxs_ap.space == MemorySpace.SBUF
    assert out_ap.space == MemorySpace.SBUF
    assert ap_utils.ap_is_contiguous(in_ap.ap[1:])
    assert ap_utils.ap_is_contiguous(out_ap.ap[1:])
    assert ap_utils.ap_is_contiguous(idxs_ap.ap[1:])
    assert in_ap.ap[-1][1] == out_ap.ap[-1][1] == elem_size
    assert out_ap.ap[0][1] * out_ap.ap[1][1] == round_up_to_multiple(
        num_idxs, 128)
    assert in_ap.ap[0][0] == elem_step
    stride_bytes = elem_step * mybir.dt.size(in_ap.dtype)
    stride_bytes_256 = exact_div(stride_bytes, 256)
    assert stride_bytes_256 < 256
    _in_ap = g.lower_ap_dma(in_ap, for_custom_bir_dma=True)
    _idxs_ap = g.lower_ap(idxs_ap)
    _out_ap = g.lower_ap(out_ap)
    return g.add_instruction(
        mybir.InstDMAGatherAnt(
            name=g.bass.get_next_instruction_name(),
            ins=[*_in_ap, _idxs_ap, g.lower_val_access(g.to_reg(num_idxs))],
            outs=[_out_ap],
            transpose=False,
            num_idxs=num_idxs,
            elem_size=elem_size,
            stride_bytes_256=stride_bytes_256,
            gen_mode=0,
            single_packet=False,
            queue_num=0,
            sbuf_tokens_per_rank=0,
            sbuf_free_dim_per_rank=0,
            sbuf_free_dim_pad_per_rank=0,
            sbuf_byte_offset=0,
        ))


# ---------------------------------------------------------------- launch A
def build_launch_a():
    nc = bacc.Bacc("TRN2", target_bir_lowering=False, debug=False,
                   num_devices=NCORES)
    hT = nc.dram_tensor("hT", [2, P, GPAD], _bf16, kind="ExternalInput")
    W = nc.dram_tensor("W", [IN_DIM, OUT_DIM], _bf16, kind="ExternalInput")
    normo = nc.dram_tensor("normo", [P, NG], _f32, kind="ExternalInput")
    m = nc.dram_tensor("m", [P, NG * OUT_DIM], _bf16, kind="ExternalOutput")

    LBLK = 14   # groups per load/store slice, for DMA/compute overlap

    with tile.TileContext(nc) as tc:
        with tc.tile_pool(name="const", bufs=1) as cpool, \
                tc.tile_pool(name="psum", bufs=8, space="PSUM") as psum:
            w0 = cpool.tile([P, OUT_DIM], _bf16, tag="w0")
            w1 = cpool.tile([P, OUT_DIM], _bf16, tag="w1")
            nc.sync.dma_start(out=w0[:], in_=W[0:P, :])
            nc.sync.dma_start(out=w1[:], in_=W[P:2 * P, :])
            norm = cpool.tile([P, NG], _f32, tag="norm")
            nc.sync.dma_start(out=norm[:], in_=normo[:, :])

            l0 = cpool.tile([P, GPAD], _bf16, tag="l0")
            l1 = cpool.tile([P, GPAD], _bf16, tag="l1")
            m_all = cpool.tile([P, NG * OUT_DIM], _bf16, tag="mall")
            for g0 in range(0, NG, LBLK):
                nb = min(LBLK, NG - g0)
                nc.sync.dma_start(out=l0[:, g0 * P:(g0 + nb) * P],
                                  in_=hT[0, :, g0 * P:(g0 + nb) * P])
                nc.sync.dma_start(out=l1[:, g0 * P:(g0 + nb) * P],
                                  in_=hT[1, :, g0 * P:(g0 + nb) * P])
            for g0 in range(0, NG, LBLK):
                nb = min(LBLK, NG - g0)
                for g in range(g0, g0 + nb):
                    acc = psum.tile([P, OUT_DIM], _f32, tag="acc")
                    nc.tensor.matmul(acc[:], l0[:, g * P:(g + 1) * P], w0[:],
                                     start=True, stop=False)
                    nc.tensor.matmul(acc[:], l1[:, g * P:(g + 1) * P], w1[:],
                                     start=False, stop=True)
                    if g % 2 == 0:
                        nc.vector.tensor_scalar_mul(
                            out=m_all[:, g * OUT_DIM:(g + 1) * OUT_DIM],
                            in0=acc[:], scalar1=norm[:, g:g + 1])
                    else:
                        nc.scalar.activation(
                            out=m_all[:, g * OUT_DIM:(g + 1) * OUT_DIM],
                            in_=acc[:],
                            func=mybir.ActivationFunctionType.Identity,
                            scale=norm[:, g:g + 1])
                nc.sync.dma_start(
                    out=m[:, g0 * OUT_DIM:(g0 + nb) * OUT_DIM],
                    in_=m_all[:, g0 * OUT_DIM:(g0 + nb) * OUT_DIM])
    nc.compile()
    return nc


# ---------------------------------------------------------------- launch B
def build_launch_b(meta):
    nc = bacc.Bacc("TRN2", target_bir_lowering=False, debug=False,
                   num_devices=NCORES)
    tabs = [nc.dram_tensor(f"t{q}", [TROWS, TSTRIDE], _f8,
                           kind="ExternalInput") for q in range(NT)]
    gidx = nc.dram_tensor("gidx", [P, meta["tot_idx_cols"]], _i16,
                          kind="ExternalInput")
    lrelp = nc.dram_tensor("lrelp", [P, meta["tot_lp"]], _bf16,
                           kind="ExternalInput")
    lrels = nc.dram_tensor("lrels", [P, max(meta["tot_ls"], 1)], _bf16,
                           kind="ExternalInput")
    normi = nc.dram_tensor("normi", [P, NG], _f32, kind="ExternalInput")
    brep = nc.dram_tensor("brep", [P, OUT_DIM], _f32, kind="ExternalInput")
    # iota[p, t, c] = t (bf16) — compare target for the one-hot builds
    nmax = meta["nmax"]
    iota = nc.dram_tensor("iota", [P, WIN, nmax], _bf16, kind="ExternalInput")
    out = nc.dram_tensor("out", [P, NG * OUT_DIM], _bf16,
                         kind="ExternalOutput")

    # one act-table set covering Identity+Exp+Ln, loaded once up front —
    # otherwise the table-load pass alternates exp/ln sets per group
    from concourse.hw_specs import get_activation_tables
    need = {mybir.ActivationFunctionType.Identity,
            mybir.ActivationFunctionType.Exp,
            mybir.ActivationFunctionType.Ln}
    set_id = next(i for i, (_, fns) in
                  enumerate(get_activation_tables(nc.m.arch).items())
                  if need <= fns)

    with tile.TileContext(nc) as tc:
        with tc.tile_pool(name="const", bufs=1) as cpool, \
                tc.tile_pool(name="gath", bufs=4) as gpool, \
                tc.tile_pool(name="meta", bufs=3) as mpool, \
                tc.tile_pool(name="ohp", bufs=4) as opool, \
                tc.tile_pool(name="ohs", bufs=4) as spool, \
                tc.tile_pool(name="epi", bufs=4) as epool, \
                tc.tile_pool(name="psum", bufs=8, space="PSUM") as psum:
            nc.scalar.add_instruction(mybir.InstLoadActFuncSet(
                name=nc.get_next_instruction_name(), ins=[], outs=[],
                act_func_set_id=set_id))
            bt = cpool.tile([P, OUT_DIM], _f32, tag="b")
            it = cpool.tile([P, WIN, nmax], _bf16, tag="iota")
            norm = cpool.tile([P, NG], _f32, tag="norm")
            nc.sync.dma_start(out=bt[:], in_=brep[:, :])
            nc.sync.dma_start(out=it[:], in_=iota[:, :, :])
            nc.sync.dma_start(out=norm[:], in_=normi[:, :])

            # per-group softmax state; fin staged for chunked stores
            s_all = cpool.tile([P, NG], _f32, tag="sall")
            ls_all = cpool.tile([P, NG], _f32, tag="lsall")
            fin_all = cpool.tile([P, NG * OUT_DIM], _bf16, tag="fin")
            stored = 0

            for rnd in meta["rounds"]:
                gt = gpool.tile([P, rnd["ncols"], OUT_DIM], _f8, tag="gt")
                icol = 0
                for q in range(NT):
                    nq = rnd["q_num"][q]
                    if nq == 0:
                        continue
                    c0 = rnd["q_col0"][q]
                    # per-q idx slice: gather q starts as soon as its own
                    # indices land, instead of waiting for the whole round
                    ixq = mpool.tile([P, nq // 16], _i16, tag=f"ix{q}")
                    nc.sync.dma_start(
                        out=ixq[:],
                        in_=gidx[:, rnd["idx_off"] + icol:
                                 rnd["idx_off"] + icol + nq // 16])
                    dma_gather_sub256(
                        nc.gpsimd,
                        gt[:, c0:c0 + nq // P, :],
                        tabs[q][:, 0:OUT_DIM],
                        ixq[:, :],
                        nq, OUT_DIM, TSTRIDE)
                    icol += nq // 16
                lpt = mpool.tile([P, rnd["lp_n"]], _bf16, tag="lp")
                nc.sync.dma_start(
                    out=lpt[:],
                    in_=lrelp[:, rnd["lp_off"]:rnd["lp_off"] + rnd["lp_n"]])
                ls_n = rnd["ls_n"]
                if ls_n:
                    lst = mpool.tile([P, ls_n], _bf16, tag="ls")
                    nc.sync.dma_start(
                        out=lst[:],
                        in_=lrels[:, rnd["ls_off"]:rnd["ls_off"] + ls_n])

                for g in rnd["groups"]:
                    pg = rnd["pergroup"][g]
                    npri, nsec = pg["npri"], pg["nsec"]
                    ohp = opool.tile([P, WIN, meta["max_pri"]], _bf16,
                                     tag="ohp")
                    nc.vector.tensor_tensor(
                        out=ohp[:, :, 0:npri],
                        in0=_expand_mid(
                            lpt[:, pg["lp_g0"]:pg["lp_g0"] + npri], WIN),
                        in1=it[:, :, 0:npri],
                        op=mybir.AluOpType.is_equal)
                    if nsec:
                        ohs = spool.tile([P, WIN, meta["max_sec"]], _bf16,
                                         tag="ohs")
                        nc.vector.tensor_tensor(
                            out=ohs[:, :, 0:nsec],
                            in0=_expand_mid(
                                lst[:, pg["ls_g0"]:pg["ls_g0"] + nsec], WIN),
                            in1=it[:, :, 0:nsec],
                            op=mybir.AluOpType.is_equal)
                    acc = psum.tile([P, OUT_DIM], _f32, tag="acc")
                    for (gtcol, kind, ohidx, base, st, sp) in pg["jobs"]:
                        oh = ohp if kind == 0 else ohs
                        nc.tensor.matmul(
                            acc[base:base + WIN, :], oh[:, :, ohidx],
                            gt[:, gtcol, :], start=st, stop=sp,
                            tile_position=(0, base))

                    x = epool.tile([P, OUT_DIM], _f32, tag="x")
                    if meta["skip_max"]:
                        # logits are small (bound checked on host): skip the
                        # max-subtraction; exp reads psum directly with the
                        # norm folded into the activation scale, x computed
                        # in parallel on DVE
                        e = epool.tile([P, OUT_DIM], _f32, tag="e")
                        nc.scalar.activation(
                            out=e[:], in_=acc[:],
                            func=mybir.ActivationFunctionType.Exp,
                            scale=norm[:, g:g + 1],
                            accum_out=s_all[:, g:g + 1])
                        nc.vector.tensor_scalar_mul(
                            out=x[:], in0=acc[:], scalar1=norm[:, g:g + 1])
                        if meta["has_bias"]:
                            nc.vector.tensor_add(out=x[:], in0=x[:],
                                                 in1=bt[:])
                    else:
                        nc.scalar.activation(
                            out=x[:], in_=acc[:],
                            func=mybir.ActivationFunctionType.Identity,
                            scale=norm[:, g:g + 1])
                        if meta["has_bias"]:
                            nc.vector.tensor_add(out=x[:], in0=x[:],
                                                 in1=bt[:])
                        nmx = epool.tile([P, 1], _f32, tag="nmx")
                        nc.vector.tensor_reduce(out=nmx[:], in_=x[:],
                                                axis=mybir.AxisListType.X,
                                                op=mybir.AluOpType.max,
                                                negate=True)
                        e = epool.tile([P, OUT_DIM], _f32, tag="e")
                        nc.scalar.activation(
                            out=e[:], in_=x[:],
                            func=mybir.ActivationFunctionType.Exp,
                            bias=nmx[:, :1], accum_out=s_all[:, g:g + 1])
                        nc.vector.tensor_scalar_add(
                            out=x[:], in0=x[:], scalar1=nmx[:, :1])
                    # finalize log-softmax inline (overlaps later rounds)
                    nc.scalar.activation(
                        out=ls_all[:, g:g + 1], in_=s_all[:, g:g + 1],
                        func=mybir.ActivationFunctionType.Ln)
                    nc.vector.tensor_scalar_sub(
                        out=fin_all[:, g * OUT_DIM:(g + 1) * OUT_DIM],
                        in0=x[:], scalar1=ls_all[:, g:g + 1])
                    if g + 1 - stored >= 24 or g == NG - 1:
                        nc.sync.dma_start(
                            out=out[:, stored * OUT_DIM:(g + 1) * OUT_DIM],
                            in_=fin_all[:, stored * OUT_DIM:
                                        (g + 1) * OUT_DIM])
                        stored = g + 1
    nc.compile()
    return nc


# ------------------------------------------------------------- host prep
def _wrap_idx16(flat):
    """int16 index list -> [128, len/16] wrapped layout (16-partition groups,
    replicated across the 8 gpsimd cores)."""
    n = len(flat)
    s = n // 16
    arr = np.empty((P, s), dtype=np.int16)
    blk = flat.reshape(s, 16).T  # [16, s]
    for grp in range(8):
        arr[grp * 16:(grp + 1) * 16, :] = blk
    return arr


def prepare(h, W, b, edges):
    h = np.asarray(h, dtype=np.float32)
    W = np.asarray(W, dtype=np.float32)
    b = np.asarray(b, dtype=np.float32)
    src = np.asarray(edges[0], dtype=np.int64)
    dst = np.asarray(edges[1], dtype=np.int64)

    out_deg = np.bincount(src, minlength=N_NODES).astype(np.float32)
    in_deg = np.bincount(dst, minlength=N_NODES).astype(np.float32)
    norm_src = np.maximum(out_deg, 1.0) ** -0.5
    norm_dst = np.maximum(in_deg, 1.0) ** -0.5

    # global m-table row for each src node (padded per-core layout)
    score = src // G
    mrow = score * GPAD + (src - score * G)
    qtab = mrow // TROWS
    lrow = (mrow - qtab * TROWS).astype(np.int16)

    dcore = dst // G
    dloc = dst - dcore * G
    grp = dloc // P
    slot = (dloc - grp * P).astype(np.int64)

    # per-core sorted edge structure
    per_core = []
    counts = np.zeros((NCORES, NG, NT), dtype=np.int64)
    for c in range(NCORES):
        msk = dcore == c
        gq = grp[msk] * NT + qtab[msk]
        order = np.argsort(gq * P * 2 + slot[msk], kind="stable")
        per_core.append(dict(gq=gq[order], slot=slot[msk][order],
                             lrow=lrow[msk][order]))
        counts[c] = np.bincount(gq, minlength=NG * NT).reshape(NG, NT)

    # round / row layout: per round, per q one packed region. Each CORE
    # packs its own group runs back-to-back inside the region (row->group
    # boundaries are core-specific); the region is sized to the max core
    # total, padded to a 128 multiple. The per-(column, group) masking in
    # the lrel arrays absorbs the per-core boundary differences.
    rounds_groups = []
    g0 = 0
    for sz in ROUND_SIZES:
        rounds_groups.append(list(range(g0, g0 + sz)))
        g0 += sz

    region_rows = {}                                  # (ri, q) -> (row0, n)
    round_row0 = []
    rows_cum = 0
    for ri, gs in enumerate(rounds_groups):
        round_row0.append(rows_cum)
        for q in range(NT):
            rcap = int(counts[:, gs, q].sum(axis=1).max())
            rcap = ((rcap + P - 1) // P) * P
            region_rows[(ri, q)] = (rows_cum, rcap)
            rows_cum += rcap
    tot_rows = rows_cum
    tot_cols = tot_rows // P

    # per-core run starts within regions
    run_row0_c = np.zeros((NCORES, NG, NT), dtype=np.int64)
    for ri, gs in enumerate(rounds_groups):
        for q in range(NT):
            r0 = region_rows[(ri, q)][0]
            for c in range(NCORES):
                cum = r0
                for g in gs:
                    run_row0_c[c, g, q] = cum
                    cum += counts[c, g, q]

    # per-core row arrays (slot per row; table idx per row; group per row)
    slots_rows = np.full((NCORES, tot_rows), PAD_SLOT, dtype=np.float32)
    idx_rows = np.zeros((NCORES, tot_rows), dtype=np.int16)
    rowg_c = np.full((NCORES, tot_rows), -1, dtype=np.int64)
    for c in range(NCORES):
        pc = per_core[c]
        cnt = counts[c].reshape(-1)
        cum = np.concatenate([[0], np.cumsum(cnt)])
        rank = np.arange(len(pc["gq"])) - np.repeat(cum[:-1], cnt)
        pos = np.repeat(run_row0_c[c].reshape(-1), cnt) + rank
        slots_rows[c, pos] = pc["slot"]
        idx_rows[c, pos] = pc["lrow"]
        rowg_c[c, pos] = pc["gq"] // NT

    rowg_col_c = rowg_c.reshape(NCORES, tot_cols, P)

    # windows hit per (column, group) — union over cores
    scol = slots_rows.reshape(NCORES, tot_cols, P)
    win_col = (scol // WIN).astype(np.int64)          # PAD -> >= NWIN

    # build per-column segment -> hit-window sets (union over cores)
    colseg = []                                       # col -> [(g, [wins])]
    for col in range(tot_cols):
        seen = {}
        for c in range(NCORES):
            rg = rowg_col_c[c, col]
            wc = win_col[c, col]
            for g in np.unique(rg):
                if g < 0:
                    continue
                msk = rg == g
                for wn in np.unique(wc[msk]):
                    if wn < NWIN:
                        seen.setdefault(int(g), set()).add(int(wn))
        colseg.append(sorted((g, sorted(w)) for g, w in seen.items()))

    # job construction (uniform): per column, the first (g, win) is primary
    max_pri = 0
    max_sec = 0
    meta_rounds = []
    idx_off = 0
    lp_off = 0
    ls_off = 0
    lp_entries = []   # (col, g, winbase) per lrelp column, global order
    ls_entries = []   # (col, g, winbase) per lrels column, global order
    for ri, gs in enumerate(rounds_groups):
        r_col0 = round_row0[ri] // P
        q_num = [region_rows[(ri, q)][1] for q in range(NT)]
        q_col0 = [(region_rows[(ri, q)][0] - round_row0[ri]) // P
                  for q in range(NT)]
        ncols_r = sum(q_num) // P
        idx_cols = sum(q_num) // 16

        # per group: primary columns (in col order) and secondary jobs
        prim = {g: [] for g in gs}    # g -> [(col, win)]
        sec = {g: [] for g in gs}     # g -> [(col, win)]
        for col in range(r_col0, r_col0 + ncols_r):
            segs = colseg[col]
            if not segs:
                continue
            first = True
            for (g, wins) in segs:
                for wn in wins:
                    if first:
                        prim[g].append((col, wn))
                        first = False
                    else:
                        sec[g].append((col, wn))
        pergroup = {}
        lp_n = 0
        ls_n = 0
        for g in gs:
            # ensure every window has at least one job (psum start/stop)
            have = {wn for (_, wn) in prim[g]} | {wn for (_, wn) in sec[g]}
            for wn in range(NWIN):
                if wn not in have:
                    anchor = prim[g][0][0] if prim[g] else r_col0
                    sec[g].append((anchor, wn))
            npri, nsec = len(prim[g]), len(sec[g])
            win_jobs = {wn: [] for wn in range(NWIN)}
            for k, (col, wn) in enumerate(prim[g]):
                win_jobs[wn].append((col, 0, k))
            for j, (col, wn) in enumerate(sec[g]):
                win_jobs[wn].append((col, 1, j))
            jobs = []
            for wn in range(NWIN):
                wj = win_jobs[wn]
                for i, (col, kind, ohidx) in enumerate(wj):
                    jobs.append((col - r_col0, kind, ohidx, wn * WIN,
                                 i == 0, i == len(wj) - 1))
            pergroup[g] = dict(npri=npri, nsec=nsec,
                               lp_g0=lp_n, ls_g0=ls_n, jobs=jobs)
            lp_entries.extend((col, g, wn * WIN) for (col, wn) in prim[g])
            ls_entries.extend((col, g, wn * WIN) for (col, wn) in sec[g])
            lp_n += npri
            ls_n += nsec
            max_pri = max(max_pri, npri)
            max_sec = max(max_sec, nsec)
        meta_rounds.append(dict(
            groups=gs, q_num=q_num, q_col0=q_col0, ncols=ncols_r,
            idx_cols=idx_cols, idx_off=idx_off,
            lp_off=lp_off, lp_n=lp_n, ls_off=ls_off, ls_n=ls_n,
            pergroup=pergroup))
        idx_off += idx_cols
        lp_off += lp_n
        ls_off += ls_n

    tot_lp = lp_off
    tot_ls = ls_off
    nmax = max(max_pri, max_sec, 1)
    meta = dict(rounds=meta_rounds, tot_idx_cols=idx_off, tot_lp=tot_lp,
                tot_ls=tot_ls, max_pri=max_pri, max_sec=max(max_sec, 1),
                nmax=nmax, has_bias=bool(np.any(b)))
    # logits bound is re-checked in kernel() after launch A produces m;
    # skip-max requires no bias (exp folds only the scale)
    meta["skip_max"] = not meta["has_bias"]

    # per-core gidx / lrel arrays (rows of other groups masked to PAD)
    lp_cols = np.asarray([c for (c, _, _) in lp_entries], dtype=np.int64)
    lp_g = np.asarray([g for (_, g, _) in lp_entries], dtype=np.int64)
    lp_base = np.asarray([bb for (_, _, bb) in lp_entries], dtype=np.int64)
    if tot_ls:
        ls_cols = np.asarray([c for (c, _, _) in ls_entries], dtype=np.int64)
        ls_g = np.asarray([g for (_, g, _) in ls_entries], dtype=np.int64)
        ls_base = np.asarray([bb for (_, _, bb) in ls_entries],
                             dtype=np.int64)
    gidx_cores = []
    lrelp_cores = []
    lrels_cores = []
    for c in range(NCORES):
        gidx_cores.append(_wrap_idx16(idx_rows[c]))
        sc = scol[c]                                   # [tot_cols, P]
        rgc = rowg_col_c[c]
        gm = rgc[lp_cols] == lp_g[:, None]             # [nlp, P]
        lp = np.where(gm, sc[lp_cols] - lp_base[:, None], PAD_SLOT).T
        lrelp_cores.append(np.ascontiguousarray(lp)
                           .astype(ml_dtypes.bfloat16))
        if tot_ls:
            gms = rgc[ls_cols] == ls_g[:, None]
            lsv = np.where(gms, sc[ls_cols] - ls_base[:, None], PAD_SLOT).T
            lrels_cores.append(np.ascontiguousarray(lsv)
                               .astype(ml_dtypes.bfloat16))
        else:
            lrels_cores.append(
                np.full((P, 1), PAD_SLOT, dtype=ml_dtypes.bfloat16))

    # norm tiles [128, NG] (partition = node % 128 within group)
    def norm_tile(nrm):
        tiles = []
        for c in range(NCORES):
            d = np.ones(GPAD, dtype=np.float32)
            d[:G] = nrm[c * G:(c + 1) * G]
            tiles.append(d.reshape(NG, P).T.copy())
        return tiles

    normo_tiles = norm_tile(norm_src)
    normi_tiles = norm_tile(norm_dst)

    hT_cores = []
    h16 = h.astype(ml_dtypes.bfloat16)
    for c in range(NCORES):
        hp = np.zeros((GPAD, IN_DIM), dtype=ml_dtypes.bfloat16)
        hp[:G] = h16[c * G:(c + 1) * G]
        # [2, 128, GPAD]: k-halves, contiguous along nodes for wide DMAs
        ht = np.ascontiguousarray(hp.T.reshape(2, P, GPAD))
        hT_cores.append(ht)

    brep = np.broadcast_to(b, (P, OUT_DIM)).copy()
    iota = np.broadcast_to(
        np.arange(WIN, dtype=np.float32)[None, :, None],
        (P, WIN, nmax)).astype(ml_dtypes.bfloat16).copy()

    return dict(meta=meta, gidx=gidx_cores, lrelp=lrelp_cores,
                lrels=lrels_cores, normo=normo_tiles, normi=normi_tiles,
                hT=hT_cores, W=W.astype(ml_dtypes.bfloat16), brep=brep,
                iota=iota,
                max_sqrt_indeg=float(np.sqrt(np.maximum(in_deg, 1.0)).max()))


_cache = {}


def _get_programs(meta):
    if "a" not in _cache:
        _cache["a"] = build_launch_a()
    if "b" not in _cache:
        _cache["b"] = build_launch_b(meta)
    return _cache["a"], _cache["b"]


def run_launch_a(nc_a, prep):
    in_maps = [{"hT": prep["hT"][c], "W": prep["W"],
                "normo": prep["normo"][c]} for c in range(NCORES)]
    res = run_bass_kernel_spmd(nc_a, in_maps, list(range(NCORES)))
    return [r["m"] for r in res.results]


def make_tabs(m_shards):
    """m_shards: per-core [128, NG*64] bf16 -> 4 fp8 sub-tables with 256B
    row stride, 64B payload."""
    m_full = np.empty((NCORES * GPAD, OUT_DIM), dtype=np.float32)
    for c, ms in enumerate(m_shards):
        # node c*GPAD + g*128 + p  <- ms[p, g*64:(g+1)*64]
        m_full[c * GPAD:(c + 1) * GPAD] = (
            ms.astype(np.float32).reshape(P, NG, OUT_DIM)
            .transpose(1, 0, 2).reshape(GPAD, OUT_DIM))
    m8 = m_full.astype(ml_dtypes.float8_e4m3)
    tabs = {}
    for q in range(NT):
        t = np.zeros((TROWS, TSTRIDE), dtype=ml_dtypes.float8_e4m3)
        t[:, :OUT_DIM] = m8[q * TROWS:(q + 1) * TROWS]
        tabs[f"t{q}"] = t
    return tabs


def run_launch_b(nc_b, prep, m_shards):
    tabs = make_tabs(m_shards)
    in_maps = [dict(tabs, gidx=prep["gidx"][c], lrelp=prep["lrelp"][c],
                    lrels=prep["lrels"][c], normi=prep["normi"][c],
                    brep=prep["brep"], iota=prep["iota"])
               for c in range(NCORES)]
    res = run_bass_kernel_spmd(nc_b, in_maps, list(range(NCORES)))
    outs = []
    for r in res.results:
        fin = r["out"].astype(np.float32).reshape(P, NG, OUT_DIM)
        outs.append(fin.transpose(1, 0, 2).reshape(GPAD, OUT_DIM)[:G])
    return np.concatenate(outs, axis=0)


def kernel(h, W, b, edges):
    prep = prepare(h, W, b, edges)
    meta = prep["meta"]
    if "a" not in _cache:
        _cache["a"] = build_launch_a()
    m_shards = run_launch_a(_cache["a"], prep)
    if meta["skip_max"]:
        # rigorous overflow check for the no-max log-softmax: |logit| <=
        # max|m| * max_d sqrt(indeg_d); exp must stay finite in fp32
        maxm = max(np.abs(ms.astype(np.float32)).max() for ms in m_shards)
        if maxm * prep["max_sqrt_indeg"] >= 60.0:
            meta["skip_max"] = False
            _cache.pop("b", None)
    if "b" not in _cache:
        _cache["b"] = build_launch_b(meta)
    out = run_launch_b(_cache["b"], prep, m_shards)
    return out.astype(np.float32)


# revision 41
# speedup vs baseline: 1.0069x; 1.0069x over previous
"""GraphConv (DGL norm='both') + log_softmax on 8 Trainium2 NeuronCores.

Strategy (per sharding hint): partition nodes across the 8 cores by range.
  Launch A (per core): project its 12500-node slice m = (h @ W) * out_deg^-1/2
  in bf16, staged as one [128, NG*64] tile, chunked loads/stores for overlap.
  Host: quantize m to fp8e4m3 and lay it out as 4 sub-tables (int16 gather
  index limit) with 256B row stride / 64B payload.
  Launch B (per core): for its 12500 dst nodes, dma_gather the fp8 rows of
  all in-edge sources (64B payload per descriptor), build narrow bf16
  one-hot selectors per 128-row chunk (edges pre-sorted by dst slot), and
  segment-sum via mixed bf16xfp8 matmuls accumulating into per-window PSUM
  partition ranges; then norm/bias/log_softmax, single bf16 store.

Degrees, norms and all edge metadata (sorting, capacities, window/job
structure) are sharding-prep computed on the host (numpy); all FLOPs on
h/W/b/m run on device.
"""

import numpy as np
import ml_dtypes

import concourse.bass as bass
import concourse.bacc as bacc
import concourse.mybir as mybir
import concourse.tile as tile
from concourse import ap_utils
from concourse.bass import AP, MemorySpace, round_up_to_multiple, exact_div
from concourse.bass_utils import run_bass_kernel_spmd

P = 128
N_NODES = 100000
N_EDGES = 3200000
IN_DIM = 256
OUT_DIM = 64
NCORES = 8
G = N_NODES // NCORES            # 12500 nodes per core
NG = (G + P - 1) // P            # 98 groups of 128 dst nodes (last has 84)
GPAD = NG * P                    # 12544
NT = 4                           # gather sub-tables (int16 index limit)
TROWS = (NCORES * GPAD) // NT    # 25088 rows per sub-table
TSTRIDE = 256                    # fp8 bytes per table row (256B stride req)
# round sizes: small first/last rounds shrink pipeline startup/tail; the
# middle rounds amortize the per-gather SWDGE fixed cost
ROUND_SIZES = [2, 4] + [8] * 11 + [2, 1, 1]
assert sum(ROUND_SIZES) == 98
WIN = 32                         # dst-slot window width (psum partition tile)
NWIN = P // WIN
PAD_SLOT = 200.0                 # slot value for padded rows (no window match)

_f32 = mybir.dt.float32
_bf16 = mybir.dt.bfloat16
_f8 = mybir.dt.float8e4
_i16 = mybir.dt.int16


def _expand_mid(ap, n):
    """[P, C] AP -> [P, n, C] AP repeating each partition row n times
    (middle broadcast keeps the last dim packed, so DVE 2x mode applies)."""
    (ps, pc), (cs, cc) = ap.ap[0], ap.ap[1]
    return AP(ap.tensor, ap.offset, [[ps, pc], [0, n], [cs, cc]])


def dma_gather_sub256(g, out_ap, in_ap, idxs_ap, num_idxs, elem_size,
                      elem_step):
    """bass.BassGpSimd.dma_gather (non-transpose, DRAM src, gen_mode=0) with
    payload < 256B: only the table row stride must be a 256B multiple (the
    descriptor stride field is encoded in 256B units); elem_size is a free
    byte count. Verified bit-exact on device for 64B/128B payloads."""
    assert idxs_ap.dtype == mybir.dt.int16
    assert in_ap.dtype == out_ap.dtype
    assert in_ap.space == MemorySpace.DRAM
    assert idxs_ap.space == MemorySpace.SBUF
    assert out_ap.space == MemorySpace.SBUF
    assert ap_utils.ap_is_contiguous(in_ap.ap[1:])
    assert ap_utils.ap_is_contiguous(out_ap.ap[1:])
    assert ap_utils.ap_is_contiguous(idxs_ap.ap[1:])
    assert in_ap.ap[-1][1] == out_ap.ap[-1][1] == elem_size
    assert out_ap.ap[0][1] * out_ap.ap[1][1] == round_up_to_multiple(
        num_idxs, 128)
    assert in_ap.ap[0][0] == elem_step
    stride_bytes = elem_step * mybir.dt.size(in_ap.dtype)
    stride_bytes_256 = exact_div(stride_bytes, 256)
    assert stride_bytes_256 < 256
    _in_ap = g.lower_ap_dma(in_ap, for_custom_bir_dma=True)
    _idxs_ap = g.lower_ap(idxs_ap)
    _out_ap = g.lower_ap(out_ap)
    return g.add_instruction(
        mybir.InstDMAGatherAnt(
            name=g.bass.get_next_instruction_name(),
            ins=[*_in_ap, _idxs_ap, g.lower_val_access(g.to_reg(num_idxs))],
            outs=[_out_ap],
            transpose=False,
            num_idxs=num_idxs,
            elem_size=elem_size,
            stride_bytes_256=stride_bytes_256,
            gen_mode=0,
            single_packet=False,
            queue_num=0,
            sbuf_tokens_per_rank=0,
            sbuf_free_dim_per_rank=0,
            sbuf_free_dim_pad_per_rank=0,
            sbuf_byte_offset=0,
        ))


# ---------------------------------------------------------------- launch A
def build_launch_a():
    nc = bacc.Bacc("TRN2", target_bir_lowering=False, debug=False,
                   num_devices=NCORES)
    hT = nc.dram_tensor("hT", [2, P, GPAD], _bf16, kind="ExternalInput")
    W = nc.dram_tensor("W", [IN_DIM, OUT_DIM], _bf16, kind="ExternalInput")
    normo = nc.dram_tensor("normo", [P, NG], _f32, kind="ExternalInput")
    m = nc.dram_tensor("m", [P, NG * OUT_DIM], _bf16, kind="ExternalOutput")

    LBLK = 14   # groups per load/store slice, for DMA/compute overlap

    with tile.TileContext(nc) as tc:
        with tc.tile_pool(name="const", bufs=1) as cpool, \
                tc.tile_pool(name="psum", bufs=8, space="PSUM") as psum:
            w0 = cpool.tile([P, OUT_DIM], _bf16, tag="w0")
            w1 = cpool.tile([P, OUT_DIM], _bf16, tag="w1")
            nc.sync.dma_start(out=w0[:], in_=W[0:P, :])
            nc.sync.dma_start(out=w1[:], in_=W[P:2 * P, :])
            norm = cpool.tile([P, NG], _f32, tag="norm")
            nc.sync.dma_start(out=norm[:], in_=normo[:, :])

            l0 = cpool.tile([P, GPAD], _bf16, tag="l0")
            l1 = cpool.tile([P, GPAD], _bf16, tag="l1")
            m_all = cpool.tile([P, NG * OUT_DIM], _bf16, tag="mall")
            for g0 in range(0, NG, LBLK):
                nb = min(LBLK, NG - g0)
                nc.sync.dma_start(out=l0[:, g0 * P:(g0 + nb) * P],
                                  in_=hT[0, :, g0 * P:(g0 + nb) * P])
                nc.sync.dma_start(out=l1[:, g0 * P:(g0 + nb) * P],
                                  in_=hT[1, :, g0 * P:(g0 + nb) * P])
            for g0 in range(0, NG, LBLK):
                nb = min(LBLK, NG - g0)
                for g in range(g0, g0 + nb):
                    acc = psum.tile([P, OUT_DIM], _f32, tag="acc")
                    nc.tensor.matmul(acc[:], l0[:, g * P:(g + 1) * P], w0[:],
                                     start=True, stop=False)
                    nc.tensor.matmul(acc[:], l1[:, g * P:(g + 1) * P], w1[:],
                                     start=False, stop=True)
                    if g % 2 == 0:
                        nc.vector.tensor_scalar_mul(
                            out=m_all[:, g * OUT_DIM:(g + 1) * OUT_DIM],
                            in0=acc[:], scalar1=norm[:, g:g + 1])
                    else:
                        nc.scalar.activation(
                            out=m_all[:, g * OUT_DIM:(g + 1) * OUT_DIM],
                            in_=acc[:],
                            func=mybir.ActivationFunctionType.Identity,
                            scale=norm[:, g:g + 1])
                nc.sync.dma_start(
                    out=m[:, g0 * OUT_DIM:(g0 + nb) * OUT_DIM],
                    in_=m_all[:, g0 * OUT_DIM:(g0 + nb) * OUT_DIM])
    nc.compile()
    return nc


# ---------------------------------------------------------------- launch B
def build_launch_b(meta):
    nc = bacc.Bacc("TRN2", target_bir_lowering=False, debug=False,
                   num_devices=NCORES)
    tabs = [nc.dram_tensor(f"t{q}", [TROWS, TSTRIDE], _f8,
                           kind="ExternalInput") for q in range(NT)]
    gidx = nc.dram_tensor("gidx", [P, meta["tot_idx_cols"]], _i16,
                          kind="ExternalInput")
    lrelp = nc.dram_tensor("lrelp", [P, meta["tot_lp"]], _bf16,
                           kind="ExternalInput")
    lrels = nc.dram_tensor("lrels", [P, max(meta["tot_ls"], 1)], _bf16,
                           kind="ExternalInput")
    normi = nc.dram_tensor("normi", [P, NG], _f32, kind="ExternalInput")
    brep = nc.dram_tensor("brep", [P, OUT_DIM], _f32, kind="ExternalInput")
    # iota[p, t, c] = t (bf16) — compare target for the one-hot builds
    nmax = meta["nmax"]
    iota = nc.dram_tensor("iota", [P, WIN, nmax], _bf16, kind="ExternalInput")
    out = nc.dram_tensor("out", [P, NG * OUT_DIM], _bf16,
                         kind="ExternalOutput")

    # one act-table set covering Identity+Exp+Ln, loaded once up front —
    # otherwise the table-load pass alternates exp/ln sets per group
    from concourse.hw_specs import get_activation_tables
    need = {mybir.ActivationFunctionType.Identity,
            mybir.ActivationFunctionType.Exp,
            mybir.ActivationFunctionType.Ln}
    set_id = next(i for i, (_, fns) in
                  enumerate(get_activation_tables(nc.m.arch).items())
                  if need <= fns)

    with tile.TileContext(nc) as tc:
        with tc.tile_pool(name="const", bufs=1) as cpool, \
                tc.tile_pool(name="gath", bufs=4) as gpool, \
                tc.tile_pool(name="meta", bufs=3) as mpool, \
                tc.tile_pool(name="ohp", bufs=4) as opool, \
                tc.tile_pool(name="ohs", bufs=4) as spool, \
                tc.tile_pool(name="epi", bufs=4) as epool, \
                tc.tile_pool(name="psum", bufs=8, space="PSUM") as psum:
            nc.scalar.add_instruction(mybir.InstLoadActFuncSet(
                name=nc.get_next_instruction_name(), ins=[], outs=[],
                act_func_set_id=set_id))
            bt = cpool.tile([P, OUT_DIM], _f32, tag="b")
            it = cpool.tile([P, WIN, nmax], _bf16, tag="iota")
            norm = cpool.tile([P, NG], _f32, tag="norm")
            consts_loaded = [False]

            def load_consts():
                # deferred past round 0's gather issue so the first gather's
                # index load hits the DMA engines first
                nc.sync.dma_start(out=bt[:], in_=brep[:, :])
                nc.sync.dma_start(out=it[:], in_=iota[:, :, :])
                nc.sync.dma_start(out=norm[:], in_=normi[:, :])
                consts_loaded[0] = True

            # per-group softmax state; fin staged for chunked stores
            s_all = cpool.tile([P, NG], _f32, tag="sall")
            ls_all = cpool.tile([P, NG], _f32, tag="lsall")
            fin_all = cpool.tile([P, NG * OUT_DIM], _bf16, tag="fin")
            stored = 0

            for rnd in meta["rounds"]:
                gt = gpool.tile([P, rnd["ncols"], OUT_DIM], _f8, tag="gt")
                icol = 0
                for q in range(NT):
                    nq = rnd["q_num"][q]
                    if nq == 0:
                        continue
                    c0 = rnd["q_col0"][q]
                    # per-q idx slice: gather q starts as soon as its own
                    # indices land, instead of waiting for the whole round
                    ixq = mpool.tile([P, nq // 16], _i16, tag=f"ix{q}")
                    nc.sync.dma_start(
                        out=ixq[:],
                        in_=gidx[:, rnd["idx_off"] + icol:
                                 rnd["idx_off"] + icol + nq // 16])
                    dma_gather_sub256(
                        nc.gpsimd,
                        gt[:, c0:c0 + nq // P, :],
                        tabs[q][:, 0:OUT_DIM],
                        ixq[:, :],
                        nq, OUT_DIM, TSTRIDE)
                    icol += nq // 16
                if not consts_loaded[0]:
                    load_consts()
                lpt = mpool.tile([P, rnd["lp_n"]], _bf16, tag="lp")
                nc.sync.dma_start(
                    out=lpt[:],
                    in_=lrelp[:, rnd["lp_off"]:rnd["lp_off"] + rnd["lp_n"]])
                ls_n = rnd["ls_n"]
                if ls_n:
                    lst = mpool.tile([P, ls_n], _bf16, tag="ls")
                    nc.sync.dma_start(
                        out=lst[:],
                        in_=lrels[:, rnd["ls_off"]:rnd["ls_off"] + ls_n])

                for g in rnd["groups"]:
                    pg = rnd["pergroup"][g]
                    npri, nsec = pg["npri"], pg["nsec"]
                    ohp = opool.tile([P, WIN, meta["max_pri"]], _bf16,
                                     tag="ohp")
                    nc.vector.tensor_tensor(
                        out=ohp[:, :, 0:npri],
                        in0=_expand_mid(
                            lpt[:, pg["lp_g0"]:pg["lp_g0"] + npri], WIN),
                        in1=it[:, :, 0:npri],
                        op=mybir.AluOpType.is_equal)
                    if nsec:
                        ohs = spool.tile([P, WIN, meta["max_sec"]], _bf16,
                                         tag="ohs")
                        nc.vector.tensor_tensor(
                            out=ohs[:, :, 0:nsec],
                            in0=_expand_mid(
                                lst[:, pg["ls_g0"]:pg["ls_g0"] + nsec], WIN),
                            in1=it[:, :, 0:nsec],
                            op=mybir.AluOpType.is_equal)
                    acc = psum.tile([P, OUT_DIM], _f32, tag="acc")
                    for (gtcol, kind, ohidx, base, st, sp) in pg["jobs"]:
                        oh = ohp if kind == 0 else ohs
                        nc.tensor.matmul(
                            acc[base:base + WIN, :], oh[:, :, ohidx],
                            gt[:, gtcol, :], start=st, stop=sp,
                            tile_position=(0, base))

                    x = epool.tile([P, OUT_DIM], _f32, tag="x")
                    if meta["skip_max"]:
                        # logits are small (bound checked on host): skip the
                        # max-subtraction; exp reads psum directly with the
                        # norm folded into the activation scale, x computed
                        # in parallel on DVE
                        e = epool.tile([P, OUT_DIM], _f32, tag="e")
                        nc.scalar.activation(
                            out=e[:], in_=acc[:],
                            func=mybir.ActivationFunctionType.Exp,
                            scale=norm[:, g:g + 1],
                            accum_out=s_all[:, g:g + 1])
                        nc.vector.tensor_scalar_mul(
                            out=x[:], in0=acc[:], scalar1=norm[:, g:g + 1])
                        if meta["has_bias"]:
                            nc.vector.tensor_add(out=x[:], in0=x[:],
                                                 in1=bt[:])
                    else:
                        nc.scalar.activation(
                            out=x[:], in_=acc[:],
                            func=mybir.ActivationFunctionType.Identity,
                            scale=norm[:, g:g + 1])
                        if meta["has_bias"]:
                            nc.vector.tensor_add(out=x[:], in0=x[:],
                                                 in1=bt[:])
                        nmx = epool.tile([P, 1], _f32, tag="nmx")
                        nc.vector.tensor_reduce(out=nmx[:], in_=x[:],
                                                axis=mybir.AxisListType.X,
                                                op=mybir.AluOpType.max,
                                                negate=True)
                        e = epool.tile([P, OUT_DIM], _f32, tag="e")
                        nc.scalar.activation(
                            out=e[:], in_=x[:],
                            func=mybir.ActivationFunctionType.Exp,
                            bias=nmx[:, :1], accum_out=s_all[:, g:g + 1])
                        nc.vector.tensor_scalar_add(
                            out=x[:], in0=x[:], scalar1=nmx[:, :1])
                    # finalize log-softmax inline (overlaps later rounds)
                    nc.scalar.activation(
                        out=ls_all[:, g:g + 1], in_=s_all[:, g:g + 1],
                        func=mybir.ActivationFunctionType.Ln)
                    nc.vector.tensor_scalar_sub(
                        out=fin_all[:, g * OUT_DIM:(g + 1) * OUT_DIM],
                        in0=x[:], scalar1=ls_all[:, g:g + 1])
                    if g + 1 - stored >= 24 or g == NG - 1:
                        nc.sync.dma_start(
                            out=out[:, stored * OUT_DIM:(g + 1) * OUT_DIM],
                            in_=fin_all[:, stored * OUT_DIM:
                                        (g + 1) * OUT_DIM])
                        stored = g + 1
    nc.compile()
    return nc


# ------------------------------------------------------------- host prep
def _wrap_idx16(flat):
    """int16 index list -> [128, len/16] wrapped layout (16-partition groups,
    replicated across the 8 gpsimd cores)."""
    n = len(flat)
    s = n // 16
    arr = np.empty((P, s), dtype=np.int16)
    blk = flat.reshape(s, 16).T  # [16, s]
    for grp in range(8):
        arr[grp * 16:(grp + 1) * 16, :] = blk
    return arr


def prepare(h, W, b, edges):
    h = np.asarray(h, dtype=np.float32)
    W = np.asarray(W, dtype=np.float32)
    b = np.asarray(b, dtype=np.float32)
    src = np.asarray(edges[0], dtype=np.int64)
    dst = np.asarray(edges[1], dtype=np.int64)

    out_deg = np.bincount(src, minlength=N_NODES).astype(np.float32)
    in_deg = np.bincount(dst, minlength=N_NODES).astype(np.float32)
    norm_src = np.maximum(out_deg, 1.0) ** -0.5
    norm_dst = np.maximum(in_deg, 1.0) ** -0.5

    # global m-table row for each src node (padded per-core layout)
    score = src // G
    mrow = score * GPAD + (src - score * G)
    qtab = mrow // TROWS
    lrow = (mrow - qtab * TROWS).astype(np.int16)

    dcore = dst // G
    dloc = dst - dcore * G
    grp = dloc // P
    slot = (dloc - grp * P).astype(np.int64)

    # per-core sorted edge structure
    per_core = []
    counts = np.zeros((NCORES, NG, NT), dtype=np.int64)
    for c in range(NCORES):
        msk = dcore == c
        gq = grp[msk] * NT + qtab[msk]
        order = np.argsort(gq * P * 2 + slot[msk], kind="stable")
        per_core.append(dict(gq=gq[order], slot=slot[msk][order],
                             lrow=lrow[msk][order]))
        counts[c] = np.bincount(gq, minlength=NG * NT).reshape(NG, NT)

    # round / row layout: per round, per q one packed region. Each CORE
    # packs its own group runs back-to-back inside the region (row->group
    # boundaries are core-specific); the region is sized to the max core
    # total, padded to a 128 multiple. The per-(column, group) masking in
    # the lrel arrays absorbs the per-core boundary differences.
    rounds_groups = []
    g0 = 0
    for sz in ROUND_SIZES:
        rounds_groups.append(list(range(g0, g0 + sz)))
        g0 += sz

    region_rows = {}                                  # (ri, q) -> (row0, n)
    round_row0 = []
    rows_cum = 0
    for ri, gs in enumerate(rounds_groups):
        round_row0.append(rows_cum)
        for q in range(NT):
            rcap = int(counts[:, gs, q].sum(axis=1).max())
            rcap = ((rcap + P - 1) // P) * P
            region_rows[(ri, q)] = (rows_cum, rcap)
            rows_cum += rcap
    tot_rows = rows_cum
    tot_cols = tot_rows // P

    # per-core run starts within regions
    run_row0_c = np.zeros((NCORES, NG, NT), dtype=np.int64)
    for ri, gs in enumerate(rounds_groups):
        for q in range(NT):
            r0 = region_rows[(ri, q)][0]
            for c in range(NCORES):
                cum = r0
                for g in gs:
                    run_row0_c[c, g, q] = cum
                    cum += counts[c, g, q]

    # per-core row arrays (slot per row; table idx per row; group per row)
    slots_rows = np.full((NCORES, tot_rows), PAD_SLOT, dtype=np.float32)
    idx_rows = np.zeros((NCORES, tot_rows), dtype=np.int16)
    rowg_c = np.full((NCORES, tot_rows), -1, dtype=np.int64)
    for c in range(NCORES):
        pc = per_core[c]
        cnt = counts[c].reshape(-1)
        cum = np.concatenate([[0], np.cumsum(cnt)])
        rank = np.arange(len(pc["gq"])) - np.repeat(cum[:-1], cnt)
        pos = np.repeat(run_row0_c[c].reshape(-1), cnt) + rank
        slots_rows[c, pos] = pc["slot"]
        idx_rows[c, pos] = pc["lrow"]
        rowg_c[c, pos] = pc["gq"] // NT

    rowg_col_c = rowg_c.reshape(NCORES, tot_cols, P)

    # windows hit per (column, group) — union over cores
    scol = slots_rows.reshape(NCORES, tot_cols, P)
    win_col = (scol // WIN).astype(np.int64)          # PAD -> >= NWIN

    # build per-column segment -> hit-window sets (union over cores)
    colseg = []                                       # col -> [(g, [wins])]
    for col in range(tot_cols):
        seen = {}
        for c in range(NCORES):
            rg = rowg_col_c[c, col]
            wc = win_col[c, col]
            for g in np.unique(rg):
                if g < 0:
                    continue
                msk = rg == g
                for wn in np.unique(wc[msk]):
                    if wn < NWIN:
                        seen.setdefault(int(g), set()).add(int(wn))
        colseg.append(sorted((g, sorted(w)) for g, w in seen.items()))

    # job construction (uniform): per column, the first (g, win) is primary
    max_pri = 0
    max_sec = 0
    meta_rounds = []
    idx_off = 0
    lp_off = 0
    ls_off = 0
    lp_entries = []   # (col, g, winbase) per lrelp column, global order
    ls_entries = []   # (col, g, winbase) per lrels column, global order
    for ri, gs in enumerate(rounds_groups):
        r_col0 = round_row0[ri] // P
        q_num = [region_rows[(ri, q)][1] for q in range(NT)]
        q_col0 = [(region_rows[(ri, q)][0] - round_row0[ri]) // P
                  for q in range(NT)]
        ncols_r = sum(q_num) // P
        idx_cols = sum(q_num) // 16

        # per group: primary columns (in col order) and secondary jobs
        prim = {g: [] for g in gs}    # g -> [(col, win)]
        sec = {g: [] for g in gs}     # g -> [(col, win)]
        for col in range(r_col0, r_col0 + ncols_r):
            segs = colseg[col]
            if not segs:
                continue
            first = True
            for (g, wins) in segs:
                for wn in wins:
                    if first:
                        prim[g].append((col, wn))
                        first = False
                    else:
                        sec[g].append((col, wn))
        pergroup = {}
        lp_n = 0
        ls_n = 0
        for g in gs:
            # ensure every window has at least one job (psum start/stop)
            have = {wn for (_, wn) in prim[g]} | {wn for (_, wn) in sec[g]}
            for wn in range(NWIN):
                if wn not in have:
                    anchor = prim[g][0][0] if prim[g] else r_col0
                    sec[g].append((anchor, wn))
            npri, nsec = len(prim[g]), len(sec[g])
            win_jobs = {wn: [] for wn in range(NWIN)}
            for k, (col, wn) in enumerate(prim[g]):
                win_jobs[wn].append((col, 0, k))
            for j, (col, wn) in enumerate(sec[g]):
                win_jobs[wn].append((col, 1, j))
            jobs = []
            for wn in range(NWIN):
                wj = win_jobs[wn]
                for i, (col, kind, ohidx) in enumerate(wj):
                    jobs.append((col - r_col0, kind, ohidx, wn * WIN,
                                 i == 0, i == len(wj) - 1))
            pergroup[g] = dict(npri=npri, nsec=nsec,
                               lp_g0=lp_n, ls_g0=ls_n, jobs=jobs)
            lp_entries.extend((col, g, wn * WIN) for (col, wn) in prim[g])
            ls_entries.extend((col, g, wn * WIN) for (col, wn) in sec[g])
            lp_n += npri
            ls_n += nsec
            max_pri = max(max_pri, npri)
            max_sec = max(max_sec, nsec)
        meta_rounds.append(dict(
            groups=gs, q_num=q_num, q_col0=q_col0, ncols=ncols_r,
            idx_cols=idx_cols, idx_off=idx_off,
            lp_off=lp_off, lp_n=lp_n, ls_off=ls_off, ls_n=ls_n,
            pergroup=pergroup))
        idx_off += idx_cols
        lp_off += lp_n
        ls_off += ls_n

    tot_lp = lp_off
    tot_ls = ls_off
    nmax = max(max_pri, max_sec, 1)
    meta = dict(rounds=meta_rounds, tot_idx_cols=idx_off, tot_lp=tot_lp,
                tot_ls=tot_ls, max_pri=max_pri, max_sec=max(max_sec, 1),
                nmax=nmax, has_bias=bool(np.any(b)))
    # logits bound is re-checked in kernel() after launch A produces m;
    # skip-max requires no bias (exp folds only the scale)
    meta["skip_max"] = not meta["has_bias"]

    # per-core gidx / lrel arrays (rows of other groups masked to PAD)
    lp_cols = np.asarray([c for (c, _, _) in lp_entries], dtype=np.int64)
    lp_g = np.asarray([g for (_, g, _) in lp_entries], dtype=np.int64)
    lp_base = np.asarray([bb for (_, _, bb) in lp_entries], dtype=np.int64)
    if tot_ls:
        ls_cols = np.asarray([c for (c, _, _) in ls_entries], dtype=np.int64)
        ls_g = np.asarray([g for (_, g, _) in ls_entries], dtype=np.int64)
        ls_base = np.asarray([bb for (_, _, bb) in ls_entries],
                             dtype=np.int64)
    gidx_cores = []
    lrelp_cores = []
    lrels_cores = []
    for c in range(NCORES):
        gidx_cores.append(_wrap_idx16(idx_rows[c]))
        sc = scol[c]                                   # [tot_cols, P]
        rgc = rowg_col_c[c]
        gm = rgc[lp_cols] == lp_g[:, None]             # [nlp, P]
        lp = np.where(gm, sc[lp_cols] - lp_base[:, None], PAD_SLOT).T
        lrelp_cores.append(np.ascontiguousarray(lp)
                           .astype(ml_dtypes.bfloat16))
        if tot_ls:
            gms = rgc[ls_cols] == ls_g[:, None]
            lsv = np.where(gms, sc[ls_cols] - ls_base[:, None], PAD_SLOT).T
            lrels_cores.append(np.ascontiguousarray(lsv)
                               .astype(ml_dtypes.bfloat16))
        else:
            lrels_cores.append(
                np.full((P, 1), PAD_SLOT, dtype=ml_dtypes.bfloat16))

    # norm tiles [128, NG] (partition = node % 128 within group)
    def norm_tile(nrm):
        tiles = []
        for c in range(NCORES):
            d = np.ones(GPAD, dtype=np.float32)
            d[:G] = nrm[c * G:(c + 1) * G]
            tiles.append(d.reshape(NG, P).T.copy())
        return tiles

    normo_tiles = norm_tile(norm_src)
    normi_tiles = norm_tile(norm_dst)

    hT_cores = []
    h16 = h.astype(ml_dtypes.bfloat16)
    for c in range(NCORES):
        hp = np.zeros((GPAD, IN_DIM), dtype=ml_dtypes.bfloat16)
        hp[:G] = h16[c * G:(c + 1) * G]
        # [2, 128, GPAD]: k-halves, contiguous along nodes for wide DMAs
        ht = np.ascontiguousarray(hp.T.reshape(2, P, GPAD))
        hT_cores.append(ht)

    brep = np.broadcast_to(b, (P, OUT_DIM)).copy()
    iota = np.broadcast_to(
        np.arange(WIN, dtype=np.float32)[None, :, None],
        (P, WIN, nmax)).astype(ml_dtypes.bfloat16).copy()

    return dict(meta=meta, gidx=gidx_cores, lrelp=lrelp_cores,
                lrels=lrels_cores, normo=normo_tiles, normi=normi_tiles,
                hT=hT_cores, W=W.astype(ml_dtypes.bfloat16), brep=brep,
                iota=iota,
                max_sqrt_indeg=float(np.sqrt(np.maximum(in_deg, 1.0)).max()))


_cache = {}


def _get_programs(meta):
    if "a" not in _cache:
        _cache["a"] = build_launch_a()
    if "b" not in _cache:
        _cache["b"] = build_launch_b(meta)
    return _cache["a"], _cache["b"]


def run_launch_a(nc_a, prep):
    in_maps = [{"hT": prep["hT"][c], "W": prep["W"],
                "normo": prep["normo"][c]} for c in range(NCORES)]
    res = run_bass_kernel_spmd(nc_a, in_maps, list(range(NCORES)))
    return [r["m"] for r in res.results]


def make_tabs(m_shards):
    """m_shards: per-core [128, NG*64] bf16 -> 4 fp8 sub-tables with 256B
    row stride, 64B payload."""
    m_full = np.empty((NCORES * GPAD, OUT_DIM), dtype=np.float32)
    for c, ms in enumerate(m_shards):
        # node c*GPAD + g*128 + p  <- ms[p, g*64:(g+1)*64]
        m_full[c * GPAD:(c + 1) * GPAD] = (
            ms.astype(np.float32).reshape(P, NG, OUT_DIM)
            .transpose(1, 0, 2).reshape(GPAD, OUT_DIM))
    m8 = m_full.astype(ml_dtypes.float8_e4m3)
    tabs = {}
    for q in range(NT):
        t = np.zeros((TROWS, TSTRIDE), dtype=ml_dtypes.float8_e4m3)
        t[:, :OUT_DIM] = m8[q * TROWS:(q + 1) * TROWS]
        tabs[f"t{q}"] = t
    return tabs


def run_launch_b(nc_b, prep, m_shards):
    tabs = make_tabs(m_shards)
    in_maps = [dict(tabs, gidx=prep["gidx"][c], lrelp=prep["lrelp"][c],
                    lrels=prep["lrels"][c], normi=prep["normi"][c],
                    brep=prep["brep"], iota=prep["iota"])
               for c in range(NCORES)]
    res = run_bass_kernel_spmd(nc_b, in_maps, list(range(NCORES)))
    outs = []
    for r in res.results:
        fin = r["out"].astype(np.float32).reshape(P, NG, OUT_DIM)
        outs.append(fin.transpose(1, 0, 2).reshape(GPAD, OUT_DIM)[:G])
    return np.concatenate(outs, axis=0)


def kernel(h, W, b, edges):
    prep = prepare(h, W, b, edges)
    meta = prep["meta"]
    if "a" not in _cache:
        _cache["a"] = build_launch_a()
    m_shards = run_launch_a(_cache["a"], prep)
    if meta["skip_max"]:
        # rigorous overflow check for the no-max log-softmax: |logit| <=
        # max|m| * max_d sqrt(indeg_d); exp must stay finite in fp32
        maxm = max(np.abs(ms.astype(np.float32)).max() for ms in m_shards)
        if maxm * prep["max_sqrt_indeg"] >= 60.0:
            meta["skip_max"] = False
            _cache.pop("b", None)
    if "b" not in _cache:
        _cache["b"] = build_launch_b(meta)
    out = run_launch_b(_cache["b"], prep, m_shards)
    return out.astype(np.float32)
